# revision 19
# baseline (speedup 1.0000x reference)
"""Trainium2 Bass kernel for nn_DiagTripleRCell.

Reference computation (per timestep t, elementwise over [B, D]):
    xW  = x @ W_x.T + b ;  xWd = x @ W_delta.T + b_delta        (bulk matmuls)
    v = r_h * h + xW_t ; c = tanh(v)
    delta = sigmoid(xWd_t + r_delta * h)
    h' = (1-delta)*h + delta*c
    out_t = h' * silu(h' + x_t + b_gate)

Strategy:
  - Data-parallel over batch B=16 across 8 cores (2 batch rows per core).
  - The T=2048 sequential scan is parallelized into C=16 chunks per core with
    uniform stride: chunk k's position at scan-step s is t(k,s) = LP*k + s,
    LP = (T-W)/C.  Chunk 0 starts exactly from h0 (no warmup; all S = W+LP
    steps are real); chunks k>=1 run W warmup steps from zero state before
    their LP real steps.  The recurrence contracts ~0.966/step so warmup
    error ~ 0.966^W * |h|.  No wraparound; per step one strided slice.
  - Chunks split into two groups of 8 -> two independent dependency chains
    interleave on the engines to hide cross-engine sem latency.
  - Projections (PE, fp16 inputs, fp32 PSUM accumulate) are emitted in
    first-visit order: token t is produced by chunk k = t // LP at step
    s = t mod LP, so production interleaves with scan steps and the PE
    streams just ahead of the scan instead of serializing in front of it.
  - Scan state and xw/xwd are fp16 (DVE 2x mode); bias means re-applied as
    ACT bias; per-(m-block) bias residuals folded into the PSUM->SBUF copy.
  - Gate/output phase (out = h*silu(h+x+b_gate)) runs in bulk blocks just
    behind the scan from a small circular h-history window.
"""

import numpy as np

T, B, D = 2048, 16, 1024
NCORES = 8
BS = B // NCORES          # 2 batch rows per core
MB = 8                    # number of 128-wide d blocks (D/128)
P = 128
X = MB * BS               # 16 free elems per (t): (m, b)

# tunables
C = 16                    # chunks per core
NG = 2                    # chain groups
CG = C // NG              # chunks per group
W = 192                   # warmup steps
LP = (T - W) // C         # 116 real steps per chunk (k>=1); chunk stride
S = W + LP                # 308 scan steps
WND = 16                  # h history window (steps)
GG = 8                    # gate block (steps)
PB = 10                   # production block (steps)

_cache = {}
last_exec_time_ns = None
last_res = None
last_in_maps = None

assert LP * C + W == T


def _build(scalar_r: bool, rh_imm: float, rd_imm: float,
           b_mean: float, bd_mean: float):
    import concourse.mybir as mybir
    import concourse.tile as tile
    from concourse import bacc
    from concourse.bass_types import AP

    f32 = mybir.dt.float32
    f16 = mybir.dt.float16
    AF = mybir.ActivationFunctionType
    OP = mybir.AluOpType

    nc = bacc.Bacc(None, target_bir_lowering=False)

    xT_d = nc.declare_dram_parameter("xT", [D, T * BS], f16, isOutput=False)
    XB_d = nc.declare_dram_parameter("XB", [P, S * C * X], f16, isOutput=False)
    WxT_d = nc.declare_dram_parameter("WxT", [D, D], f16, isOutput=False)
    WdT_d = nc.declare_dram_parameter("WdT", [D, D], f16, isOutput=False)
    h0_d = nc.declare_dram_parameter("h0c", [P, X], f16, isOutput=False)
    bx_d = nc.declare_dram_parameter("bxc", [D], f32, isOutput=False)
    bd_d = nc.declare_dram_parameter("bdc", [D], f32, isOutput=False)
    rh_d = nc.declare_dram_parameter("rhm", [D], f32, isOutput=False)
    rd_d = nc.declare_dram_parameter("rdm", [D], f32, isOutput=False)
    h_out = nc.declare_dram_parameter("h_out", [P, S * C * X], f32, isOutput=True)
    o_out = nc.declare_dram_parameter("o_out", [P, S * C * X], f32, isOutput=True)

    def apx(base, off, dims):
        """View `base` AP (keeps its partition dim) with custom free dims.
        off in elements of the free space; dims = [[step, count], ...]."""
        return AP(tensor=base.tensor, offset=base.offset + off,
                  ap=[list(base.ap[0])] + [list(d) for d in dims])

    with tile.TileContext(nc) as tc:
        with (
            tc.tile_pool(name="big", bufs=1) as big,
            tc.tile_pool(name="xtp", bufs=2) as xtp,
            tc.tile_pool(name="pp", bufs=8, space="PSUM") as pp,
            tc.tile_pool(name="scr", bufs=2) as scr,
            tc.tile_pool(name="gate", bufs=4) as gatep,
        ):
            xw_all = big.tile([P, T * X], f16, tag="xw")
            xwd_all = big.tile([P, T * X], f16, tag="xwd")
            wx_sb = big.tile([P, MB * D], f16, tag="wx")
            wd_sb = big.tile([P, MB * D], f16, tag="wd")
            hist = big.tile([P, WND * C * X], f16, tag="hist")
            cst = big.tile([P, 32], f32, tag="cst")
            bx_sb = cst[:, 0:MB]
            bd_sb = cst[:, MB:2 * MB]
            bm_sb = cst[:, 16:17]
            bdm_sb = cst[:, 17:18]

            nc.vector.memset(bm_sb, b_mean)
            nc.vector.memset(bdm_sb, bd_mean)
            nc.sync.dma_start(bx_sb, bx_d[:].rearrange("(m p) -> p m", p=P))
            nc.sync.dma_start(bd_sb, bd_d[:].rearrange("(m p) -> p m", p=P))
            if not scalar_r:
                rh_sb = big.tile([P, CG * X], f16, tag="rh")
                rd_sb = big.tile([P, CG * X], f16, tag="rd")
                rhm = cst[:, 18:18 + MB]
                rdm = cst[:, 18 + MB:18 + 2 * MB]
                nc.sync.dma_start(rhm, rh_d[:].rearrange("(m p) -> p m", p=P))
                nc.sync.dma_start(rdm, rd_d[:].rearrange("(m p) -> p m", p=P))
                rh4 = rh_sb[:].rearrange("p (c m b) -> p c m b", c=CG, m=MB)
                rd4 = rd_sb[:].rearrange("p (c m b) -> p c m b", c=CG, m=MB)
                for c in range(CG):
                    for bb in range(BS):
                        nc.vector.tensor_copy(rh4[:, c, :, bb], rhm)
                        nc.vector.tensor_copy(rd4[:, c, :, bb], rdm)

            nc.sync.dma_start(
                wx_sb[:].rearrange("p (k o) -> p k o", k=MB),
                WxT_d[:].rearrange("(k p) o -> p k o", p=P))
            nc.sync.dma_start(
                wd_sb[:].rearrange("p (k o) -> p k o", k=MB),
                WdT_d[:].rearrange("(k p) o -> p k o", p=P))

            hist_w = hist[:].rearrange("p (w c x) -> p w c x", w=WND, c=C)
            # state before step 0: zeros for chunks >=1, h0 for chunk 0
            nc.vector.memset(hist_w[:, WND - 1, :, :], 0.0)
            nc.sync.dma_start(hist_w[:, WND - 1, 0, :], h0_d[:])

            xw3 = xw_all[:].rearrange("p (t x) -> p t x", x=X)
            xwd3 = xwd_all[:].rearrange("p (t x) -> p t x", x=X)

            # ---------- phase 1 production ----------
            def produce_block(tag, t0, nst, nrun, rstride):
                """Produce xw/xwd for tokens t in {t0 + r*rstride + st}
                (r < nrun runs of nst steps)."""
                ntok = nst * BS
                N = nrun * ntok
                xtg = xtp.tile([P, MB * N], f16, tag="xtg", name=f"xtg{tag}")
                for k in range(MB):
                    # src rows k*128..k*128+127 of xT, cols per run
                    src = AP(tensor=xT_d[:].tensor,
                             offset=(k * P) * (T * BS) + t0 * BS,
                             ap=[[T * BS, P], [rstride * BS, nrun], [1, ntok]])
                    dst = apx(xtg[:], k * N, [[ntok, nrun], [1, ntok]])
                    nc.sync.dma_start(dst, src)
                for wsb, dstv, bias_sb in ((wx_sb, xw_all, bx_sb),
                                           (wd_sb, xwd_all, bd_sb)):
                    for m in range(MB):
                        pt = pp.tile([P, N], f32, tag="pt",
                                     name=f"pt{tag}_{m}")
                        for k in range(MB):
                            nc.tensor.matmul(
                                pt[:],
                                wsb[:, k * D + m * P:k * D + (m + 1) * P],
                                xtg[:, k * N:(k + 1) * N],
                                start=(k == 0), stop=(k == MB - 1))
                        dst = apx(dstv[:], t0 * X + m * BS,
                                  [[rstride * X, nrun], [X, nst], [1, BS]])
                        nc.scalar.activation(
                            dst,
                            pt[:].rearrange("p (r s b) -> p r s b",
                                            r=nrun, b=BS),
                            AF.Identity, bias=bias_sb[:, m:m + 1])

            # ---------- gate ----------
            XB3 = XB_d[:]
            h3 = h_out[:]
            o3 = o_out[:]

            def gate_block(tag, s0, ns, c0, cn):
                slot0 = s0 % WND
                fd = ns * cn * X
                hg = hist_w[:, slot0:slot0 + ns, c0:c0 + cn, :]
                xbb = gatep.tile([P, fd], f16, tag="g", bufs=2, name=f"xb{tag}")
                st = gatep.tile([P, fd], f16, tag="g", bufs=2, name=f"st{tag}")
                h32 = gatep.tile([P, fd], f32, tag="g2", bufs=2, name=f"h32{tag}")
                o32 = gatep.tile([P, fd], f32, tag="g2", bufs=2, name=f"o32{tag}")
                xb3 = xbb[:].rearrange("p (s c x) -> p s c x", s=ns, c=cn)
                st3 = st[:].rearrange("p (s c x) -> p s c x", s=ns, c=cn)
                h32_3 = h32[:].rearrange("p (s c x) -> p s c x", s=ns, c=cn)
                o32_3 = o32[:].rearrange("p (s c x) -> p s c x", s=ns, c=cn)
                base_off = s0 * C * X + c0 * X
                ddims = [[C * X, ns], [1, cn * X]]
                xb2 = xbb[:].rearrange("p (s y) -> p s y", s=ns)
                nc.sync.dma_start(xb2, apx(XB3, base_off, ddims))
                nc.vector.tensor_add(xb3, hg, xb3)            # g = h + xb
                nc.scalar.activation(st3, xb3, AF.Sigmoid)    # s = sig(g)
                nc.gpsimd.tensor_mul(xb3, xb3, st3)           # silu = g*s
                nc.vector.tensor_mul(o32_3, hg, xb3)          # O = h*silu
                nc.scalar.activation(h32_3, hg, AF.Copy)      # h -> fp32
                nc.sync.dma_start(apx(h3, base_off, ddims),
                                  h32[:].rearrange("p (s y) -> p s y", s=ns))
                nc.sync.dma_start(apx(o3, base_off, ddims),
                                  o32[:].rearrange("p (s y) -> p s y", s=ns))

            # ---------- scan step ----------
            def scan_step(s):
                slot_prev = (s - 1) % WND
                slot = s % WND
                for g in range(NG):
                    c0 = g * CG
                    prev = hist_w[:, slot_prev, c0:c0 + CG, :]
                    xw_s = xw3[:, LP * c0 + s:LP * c0 + s + (CG - 1) * LP + 1:LP, :]
                    xwd_s = xwd3[:, LP * c0 + s:LP * c0 + s + (CG - 1) * LP + 1:LP, :]
                    vt = scr.tile([P, 2, CG, X], f16, tag=f"sc{g}",
                                  name=f"sc{g}_{s}")
                    v = vt[:, 0]
                    u = vt[:, 1]
                    if scalar_r:
                        nc.vector.scalar_tensor_tensor(
                            v, prev, rh_imm, xw_s, OP.mult, OP.add)
                        nc.vector.scalar_tensor_tensor(
                            u, prev, rd_imm, xwd_s, OP.mult, OP.add)
                    else:
                        rh_s = rh_sb[:].rearrange("p (c x) -> p c x", c=CG)
                        rd_s = rd_sb[:].rearrange("p (c x) -> p c x", c=CG)
                        nc.vector.tensor_mul(v, prev, rh_s)
                        nc.vector.tensor_add(v, v, xw_s)
                        nc.vector.tensor_mul(u, prev, rd_s)
                        nc.vector.tensor_add(u, u, xwd_s)
                    nc.scalar.activation(v, v, AF.Tanh, bias=bm_sb)
                    nc.scalar.activation(u, u, AF.Sigmoid, bias=bdm_sb)
                    nc.gpsimd.tensor_sub(v, v, prev)          # e = c - h
                    nc.gpsimd.tensor_mul(v, u, v)             # f = delta*e
                    nc.vector.tensor_add(
                        hist_w[:, slot, c0:c0 + CG, :], prev, v)

            def gates_for(s):
                if s < W and (s + 1) % GG == 0:
                    gate_block(f"r1_{s}", s - GG + 1, GG, 0, 1)
                if s >= W and (s - W + 1) % GG == 0:
                    gate_block(f"r2_{s}", s - GG + 1, GG, 0, C)
                if s == S - 1:
                    rem1 = W % GG
                    if rem1:
                        gate_block("r1rem", W - rem1, rem1, 0, 1)
                    rem2 = (S - W) % GG
                    if rem2:
                        gate_block("r2rem", S - rem2, rem2, 0, C)

            # ---------- emission: production just ahead of scan ----------
            main_blocks = []
            s0 = 0
            while s0 < LP:
                ns = min(PB, LP - s0)
                main_blocks.append((s0, ns))
                s0 += ns
            t_tail0 = C * LP
            tail_len = (T - t_tail0) // 2

            bi = 0
            tail_done = 0
            for s in range(S):
                while bi < len(main_blocks) and main_blocks[bi][0] <= s + PB:
                    mb0, mbn = main_blocks[bi]
                    produce_block(f"m{bi}", mb0, mbn, C, LP)
                    bi += 1
                if s >= 80 and tail_done < 2:
                    produce_block(f"t{tail_done}",
                                  t_tail0 + tail_done * tail_len,
                                  tail_len, 1, tail_len)
                    tail_done += 1
                scan_step(s)
                gates_for(s)

    nc.compile()
    return nc


def kernel(x, h0, W_x, W_delta, r_h, r_delta, b, b_delta, b_gate):
    import os
    from concourse.bass_utils import run_bass_kernel_spmd

    x = np.asarray(x, dtype=np.float32)
    h0 = np.asarray(h0, dtype=np.float32)
    W_x = np.asarray(W_x, dtype=np.float32)
    W_delta = np.asarray(W_delta, dtype=np.float32)
    r_h = np.asarray(r_h, dtype=np.float32)
    r_delta = np.asarray(r_delta, dtype=np.float32)
    b = np.asarray(b, dtype=np.float32)
    b_delta = np.asarray(b_delta, dtype=np.float32)
    b_gate = np.asarray(b_gate, dtype=np.float32)

    scalar_r = bool(np.all(r_h == r_h[0]) and np.all(r_delta == r_delta[0]))
    rh_imm = float(r_h[0])
    rd_imm = float(r_delta[0])
    b_mean = float(b.mean())
    bd_mean = float(b_delta.mean())

    key = (scalar_r, rh_imm, rd_imm, b_mean, bd_mean)
    if key not in _cache:
        _cache[key] = _build(scalar_r, rh_imm, rd_imm, b_mean, bd_mean)
    nc = _cache[key]

    bx_c = b - b_mean
    bd_c = b_delta - bd_mean
    WxT16 = np.ascontiguousarray(W_x.T).astype(np.float16)
    WdT16 = np.ascontiguousarray(W_delta.T).astype(np.float16)
    XB = (x + b_gate[None, None, :]).astype(np.float32)

    in_maps = []
    for c in range(NCORES):
        sl = slice(c * BS, (c + 1) * BS)
        x_c = x[:, sl, :]
        xT = np.ascontiguousarray(
            x_c.transpose(2, 0, 1).reshape(D, T * BS)).astype(np.float16)
        XB_c = XB[:, sl, :].reshape(T, BS, MB, P)
        t_idx = (LP * np.arange(C)[None, :] + np.arange(S)[:, None])  # [S, C]
        XBs = np.ascontiguousarray(
            XB_c[t_idx].transpose(4, 0, 1, 3, 2)   # [P, S, C, MB, BS]
            .reshape(P, S * C * X)).astype(np.float16)
        in_maps.append({
            "xT": xT,
            "XB": XBs,
            "WxT": WxT16,
            "WdT": WdT16,
            "h0c": np.ascontiguousarray(
                h0[sl].reshape(BS, MB, P).transpose(2, 1, 0).reshape(P, X)
            ).astype(np.float16),
            "bxc": bx_c,
            "bdc": bd_c,
            "rhm": r_h,
            "rdm": r_delta,
        })

    trace = bool(os.environ.get("KERNEL_TRACE"))
    global last_exec_time_ns, last_res, last_in_maps
    last_in_maps = in_maps
    res = run_bass_kernel_spmd(nc, in_maps, list(range(NCORES)), trace=trace)
    last_res = res
    last_exec_time_ns = res.exec_time_ns

    out = np.empty((T, B, D), np.float32)
    h = np.empty((T + 1, B, D), np.float32)
    h[0] = h0
    for c in range(NCORES):
        sl = slice(c * BS, (c + 1) * BS)
        # scan-order [P, S, C, MB, BS] -> real cells t = LP*k + s
        ho = res.results[c]["h_out"].reshape(P, S, C, MB, BS)
        oo = res.results[c]["o_out"].reshape(P, S, C, MB, BS)
        hl = np.empty((T, BS, D), np.float32)
        ol = np.empty((T, BS, D), np.float32)
        for k in range(C):
            ss = 0 if k == 0 else W
            tt0 = LP * k + ss
            tt1 = LP * k + S
            seg_h = ho[:, ss:S, k].transpose(1, 3, 2, 0).reshape(tt1 - tt0, BS, D)
            seg_o = oo[:, ss:S, k].transpose(1, 3, 2, 0).reshape(tt1 - tt0, BS, D)
            hl[tt0:tt1] = seg_h
            ol[tt0:tt1] = seg_o
        h[1:, sl, :] = hl
        out[:, sl, :] = ol
    return out, h
